# revision 9
# baseline (speedup 1.0000x reference)
"""Bass/Trainium2 kernel for DocRE bilinear segment-reduce model.

Shapes (hardcoded): B=4, L=1024, H=768, NH=12, E=24, M=4, P=552, NL=97, BLK=64.
Sharding: 8 cores = (batch b = core//2) x (half of the 552 head-tail pairs).
One SPMD program; all per-core differences flow through input data
(gathered rows + one-hot matrices built on host from the integer indices).

Phase structure (per core):
  1. entity embeddings  eet = ln(sume @ exp(emg))                 [PE+ACT]
  2. (fused into 4 via host-weighted one-hots OHH2/OHT2)
  3. hs/ts = entity-embedding gathers per pair                    [PE]
  4. per-head attention gathers direct from amg + products + tree [PE+DVE+ACT+Pool]
  5. 1/(sum ht + 1e-5) broadcast                                  [PE+DVE]
  6. rs = seq^T @ ht (normalized on evac)                         [PE+DVE]
  7. zh/zt projections with tanh                                  [PE+ACT]
  9. bilinear: 32 strips x 12 chunks of [128, 276];
     zh-replication via broadcast-DMA (bf16, SBUF) or PE one-hot mms;
     blt = rep*ztA on DVE/Pool; accumulate logits on PE           [DMA+PE+DVE+Pool]
"""

import dataclasses
import numpy as np
import ml_dtypes

import concourse.bass as bass
import concourse.bacc as bacc
import concourse.tile as tile
from concourse import mybir
from concourse.bass_utils import run_bass_kernel_spmd

B, L, H, NH, E, M, P, NL, BLK = 4, 1024, 768, 12, 24, 4, 552, 97, 64
G = H // BLK            # 12 blocks
R = P // 2              # 276 rows per core
EM = E * M              # 96 gathered mentions
HL = NH * L             # 12288
K = H * BLK             # 49152 bilinear contraction
NSTRIP = 32             # strips of 12 chunks (i0 = 2*strip)
F32 = mybir.dt.float32
BF16 = mybir.dt.bfloat16

# strip `st` uses PE-rep (one-hot matmuls) iff st % 8 in PE_STRIPS_MOD
PE_STRIPS_MOD = (2, 5, 7)
WBT_PREFETCH = 16

_CACHE = {}


def _pe_strip(st):
    return st % 8 in PE_STRIPS_MOD


def _build_program():
    nc = bacc.Bacc("TRN2", target_bir_lowering=False, debug=False, num_devices=8)
    dp = nc.declare_dram_parameter
    EMG = dp("EMG", [EM, H], F32, isOutput=False)        # gathered mention embeddings
    SUME = dp("SUME", [EM, 128], F32, isOutput=False)    # mask one-hot (logsumexp sum)
    AMG = dp("AMG", [EM, HL], BF16, isOutput=False)      # gathered mention attn rows (h-major)
    OHH = dp("OHH", [128, R], BF16, isOutput=False)      # head-entity one-hot (ph3)
    OHT = dp("OHT", [128, R], BF16, isOutput=False)      # tail-entity one-hot (ph3)
    OHH2 = dp("OHH2", [EM, R], BF16, isOutput=False)     # mask/denom/sqrtNH one-hot (ph4)
    OHT2 = dp("OHT2", [EM, R], BF16, isOutput=False)
    SEQ = dp("SEQ", [L, H], BF16, isOutput=False)        # sequence_output[b]
    WHT = dp("WHT", [2 * H, H], BF16, isOutput=False)    # Wh.T
    WTT = dp("WTT", [2 * H, H], BF16, isOutput=False)    # Wt.T
    WBT = dp("WBT", [K, NL], BF16, isOutput=False)       # Wb.T, strip-permuted
    BHS = dp("BHS", [128, 6], F32, isOutput=False)       # bh as [128,6] per o-chunk
    BTS = dp("BTS", [128, 6], F32, isOutput=False)
    BBS = dp("BBS", [NL, 1], F32, isOutput=False)
    SEL64 = dp("SEL64", [128, 32 * 128], BF16, isOutput=False)  # PE-rep one-hots
    OUT = dp("OUT", [NL, R], F32, isOutput=True)         # logits^T

    with tile.TileContext(nc) as tc:
        with (
            tc.tile_pool(name="persist", bufs=1) as pp,
            tc.tile_pool(name="wstream", bufs=4) as wp,
            tc.tile_pool(name="wbstream", bufs=4) as wbsp,
            tc.tile_pool(name="rep", bufs=3) as repp,
            tc.tile_pool(name="blt", bufs=3) as bltp,
            tc.tile_pool(name="w4p", bufs=2) as w4p,
            tc.tile_pool(name="tsbp", bufs=3) as tsbp,
        ):
            # ---- persistent small inputs
            def load(name, ap, shape, tag, dt=F32, eng=None):
                t = pp.tile(shape, dt, tag=tag)
                (eng or nc.sync).dma_start(t[:], ap)
                return t

            emg = load("EMG", EMG[:], [EM, H], "emg")
            sume = load("SUME", SUME[:], [EM, 128], "sume")
            ohh = load("OHH", OHH[:], [128, R], "ohh", BF16)
            oht = load("OHT", OHT[:], [128, R], "oht", BF16)
            ohh2 = load("OHH2", OHH2[:], [EM, R], "ohh2", BF16)
            oht2 = load("OHT2", OHT2[:], [EM, R], "oht2", BF16)
            bhs = load("BHS", BHS[:], [128, 6], "bhs")
            bts = load("BTS", BTS[:], [128, 6], "bts")
            bbs = load("BBS", BBS[:], [NL, 1], "bbs")
            sel64 = load("SEL64", SEL64[:], [128, 32 * 128], "sel64", BF16)
            seqt = [load("SEQ", SEQ[q * 128:(q + 1) * 128, :], [128, H], f"seq{q}",
                         BF16) for q in range(8)]
            # attention rows, h-major: gather stationaries come straight from here
            amg = pp.tile([EM, HL], BF16, tag="amg", name="amg")
            for h in range(NH):
                nc.scalar.dma_start(amg[:, h * L:(h + 1) * L],
                                    AMG[:, h * L:(h + 1) * L])
            # Wb stationaries: [128, 12, NL] per strip; prefetch most
            wbt_tiles = {}
            for st in range(WBT_PREFETCH):
                t = pp.tile([128, G, NL], BF16, tag=f"wbt{st}", name=f"wbt{st}")
                nc.scalar.dma_start(
                    t[:],
                    WBT[st * 1536:(st + 1) * 1536, :].rearrange("(c p) n -> p c n", p=128))
                wbt_tiles[st] = t
            ones = pp.tile([128, 128], F32, tag="ones", name="ones")
            nc.vector.memset(ones[:], 1.0)
            onesb = pp.tile([128, 1], BF16, tag="onesb", name="onesb")
            nc.vector.memset(onesb[:], 1.0)

            # ---- phase 1: entity embeddings = ln(sum_m mask * exp(m_emb))
            expt = pp.tile([EM, H], F32, tag="expt", name="expt")
            nc.scalar.activation(expt[:], emg[:], mybir.ActivationFunctionType.Exp)
            eet = pp.tile([128, H], BF16, tag="eet", name="eet")
            with tc.tile_pool(name="ps1", bufs=2, space="PSUM") as ps1:
              for half in range(2):
                pe = ps1.tile([128, 384], F32, tag="ee_ps", name="ee_ps")
                nc.tensor.matmul(pe[:], sume[:], expt[:, half * 384:(half + 1) * 384],
                                 start=True, stop=True)
                nc.scalar.activation(eet[:, half * 384:(half + 1) * 384], pe[:],
                                     mybir.ActivationFunctionType.Ln)

            # ---- phase 3: hs^T / ts^T gathers  [128d, R] x 6
            hst, tst = [], []
            with tc.tile_pool(name="ps3", bufs=4, space="PSUM") as ps3:
              for oc in range(6):
                for si, (oh, dst_list, tag) in enumerate(
                        ((ohh, hst, "hs"), (oht, tst, "ts"))):
                    rg = ((oc * 2 + si) % 4) * 32
                    pg = ps3.tile([128, R], F32, tag="gat_ps", name="gat_ps")
                    nc.tensor.matmul(pg[:],
                                     eet[rg:rg + E, oc * 128:(oc + 1) * 128],
                                     oh[rg:rg + E, :],
                                     start=True, stop=True,
                                     tile_position=(rg, 0))
                    t = pp.tile([128, R], BF16, tag=f"{tag}{oc}", name=f"{tag}{oc}")
                    nc.scalar.copy(t[:], pg[:])
                    dst_list.append(t)

            # ---- phase 4: ht_att per l-chunk; gathers direct from amg (ph2 fused
            # into host-weighted one-hots), products in bf16, tree on DVE/Pool
            htacc = []
            with tc.tile_pool(name="ps4", bufs=2, space="PSUM") as ps4:
              for q in range(8):
                acc = pp.tile([128, R], BF16, tag=f"ht{q}", name=f"ht{q}")
                w4 = w4p.tile([128, NH, R], BF16, tag="w4", name="w4")
                for hp in range(NH // 2):
                    hh2 = ps4.tile([128, 1024], F32, tag="hh_ps", name="hh_ps")
                    tt2 = ps4.tile([128, 1024], F32, tag="tt_ps", name="tt_ps")
                    for kk in range(2):
                        h = hp * 2 + kk
                        asl = amg[:, h * L + q * 128: h * L + (q + 1) * 128]
                        nc.tensor.matmul(hh2[:, kk * 512:kk * 512 + R],
                                         asl, ohh2[:], start=True, stop=True)
                        nc.tensor.matmul(tt2[:, kk * 512:kk * 512 + R],
                                         asl, oht2[:], start=True, stop=True)
                    tview = dataclasses.replace(
                        tt2[:], ap=[tt2[:].ap[0], [512, 2], [1, R]])
                    hview = dataclasses.replace(
                        hh2[:], ap=[hh2[:].ap[0], [512, 2], [1, R]])
                    tsb = tsbp.tile([128, 2, R], BF16, tag="tsb", name="tsb")
                    nc.scalar.copy(tsb[:], tview)
                    nc.vector.tensor_tensor(w4[:, hp * 2:hp * 2 + 2, :], hview,
                                            tsb[:], mybir.AluOpType.mult)
                # tree-sum over the 12 heads (bf16, mostly DVE at 2x)
                nc.vector.tensor_add(w4[:, 0:6, :], w4[:, 0:6, :], w4[:, 6:12, :])
                nc.vector.tensor_add(w4[:, 0:3, :], w4[:, 0:3, :], w4[:, 3:6, :])
                nc.gpsimd.tensor_add(w4[:, 0, :], w4[:, 0, :], w4[:, 1, :])
                nc.vector.tensor_add(acc[:], w4[:, 0, :], w4[:, 2, :])
                htacc.append(acc)

            # ---- phase 5: 1/(sum_l ht + 1e-5), broadcast to 128 partitions
            invd = pp.tile([128, R], BF16, tag="invd", name="invd")
            with tc.tile_pool(name="ps5", bufs=1, space="PSUM") as ps5:
                psum_s = ps5.tile([1, R], F32, tag="s_ps", name="s_ps")
                for q in range(8):
                    nc.tensor.matmul(psum_s[:], onesb[:], htacc[q][:],
                                     start=(q == 0), stop=(q == 7))
                invd1 = pp.tile([1, R], F32, tag="invd1", name="invd1")
                nc.vector.tensor_scalar_add(invd1[:], psum_s[:], 1e-5)
                nc.vector.reciprocal(invd1[:], invd1[:])
                pb = ps5.tile([128, R], F32, tag="invd_ps", name="invd_ps")
                nc.tensor.matmul(pb[:], ones[0:1, :], invd1[:], start=True, stop=True)
                nc.scalar.copy(invd[:], pb[:])

            # ---- phase 6: rs^T chunks (normalization folded into evac)
            rst = []
            with tc.tile_pool(name="ps6", bufs=2, space="PSUM") as ps6:
              for dc in range(6):
                pr = ps6.tile([128, R], F32, tag="rs_ps", name="rs_ps")
                for q in range(8):
                    nc.tensor.matmul(pr[:], seqt[q][:, dc * 128:(dc + 1) * 128],
                                     htacc[q][:], start=(q == 0), stop=(q == 7))
                t = pp.tile([128, R], BF16, tag=f"rs{dc}", name=f"rs{dc}")
                nc.vector.tensor_mul(t[:], pr[:], invd[:])
                rst.append(t)

            # ---- phase 7: zh^T = tanh(Wh^T @ [hs; rs] + bh), zt likewise,
            # evacuated into consolidated [128, 6*R] tiles (rep-DMA sources)
            zht6 = pp.tile([128, 6 * R], BF16, tag="zht6", name="zht6")
            ztt6 = pp.tile([128, 6 * R], BF16, tag="ztt6", name="ztt6")
            for (wdram, inv, bias, out6, tag) in (
                    (WHT, hst, bhs, zht6, "zh"), (WTT, tst, bts, ztt6, "zt")):
              with tc.tile_pool(name=f"ps7{tag}", bufs=1, space="PSUM") as ps7:
                pps = [ps7.tile([128, R], F32, tag=f"{tag}_ps{oc}",
                                name=f"{tag}_ps{oc}") for oc in range(6)]
                for k2 in range(6):
                    wt2 = wp.tile([128, 2, H], BF16, tag="wproj", name="wproj")
                    nc.sync.dma_start(
                        wt2[:],
                        wdram[k2 * 256:(k2 + 1) * 256, :].rearrange("(j p) n -> p j n", p=128))
                    for kk in range(2):
                        kx = k2 * 2 + kk
                        rhs = inv[kx] if kx < 6 else rst[kx - 6]
                        for oc in range(6):
                            nc.tensor.matmul(pps[oc][:],
                                             wt2[:, kk, oc * 128:(oc + 1) * 128],
                                             rhs[:], start=(kx == 0), stop=(kx == 11))
                for oc in range(6):
                    nc.scalar.activation(out6[:, oc * R:(oc + 1) * R], pps[oc][:],
                                         mybir.ActivationFunctionType.Tanh,
                                         bias=bias[:, oc:oc + 1])

            # ---- phase 8: ztA[q, (s, gp, f)] = zt-row(2gp+s)*64 + q%64
            ztA = pp.tile([128, 2 * 6 * R], BF16, tag="ztA", name="ztA")
            for s in range(2):
                src = ztt6[s * 64:(s + 1) * 64, :]
                nc.sync.dma_start(ztA[0:64, s * 6 * R:(s + 1) * 6 * R], src)
                nc.sync.dma_start(ztA[64:128, s * 6 * R:(s + 1) * 6 * R], src)

            # ---- phase 9: bilinear logits^T over 32 strips x 12 chunks.
            # chunk c2 = s*6+gp of strip st: g = 2*gp+s, i = 2*st + q//64, j = q%64
            ps9_cm = tc.tile_pool(name="ps9", bufs=1, space="PSUM")
            ps9 = ps9_cm.__enter__()
            ps9r_cm = tc.tile_pool(name="ps9r", bufs=2, space="PSUM")
            ps9r = ps9r_cm.__enter__()
            lt = ps9.tile([NL, R], F32, tag="lt_ps", name="lt_ps")
            dve_i = 0
            pe_i = 0
            for st in range(NSTRIP):
                i0 = 2 * st
                if st in wbt_tiles:
                    wbc = wbt_tiles[st]
                else:
                    wbc = wbsp.tile([128, G, NL], BF16, tag="wbc", name="wbc")
                    nc.scalar.dma_start(
                        wbc[:],
                        WBT[st * 1536:(st + 1) * 1536, :].rearrange("(c p) n -> p c n", p=128))
                blt = bltp.tile([128, 12 * R], BF16, tag="blt", name="blt")
                if not _pe_strip(st):
                    # broadcast-DMA rep: 2 DMAs, each [128, 6*R] half-strip
                    rep = repp.tile([128, 12 * R], BF16, tag="rep", name="rep")
                    for s in range(2):
                        s0 = zht6[s * 64 + i0: s * 64 + i0 + 2, :]
                        sb = dataclasses.replace(
                            s0, ap=[s0.ap[0], [0, 64], [1, 6 * R]])
                        nc.sync.dma_start(rep[:, s * 6 * R:(s + 1) * 6 * R], sb)
                    if dve_i % 5 == 4:
                        nc.gpsimd.tensor_mul(blt[:], rep[:], ztA[:])
                    else:
                        nc.vector.tensor_mul(blt[:], rep[:], ztA[:])
                    dve_i += 1
                else:
                    # PE one-hot rep into PSUM pairs, multiply pairwise
                    for pg in range(6):
                        prp = ps9r.tile([128, 1024], F32, tag="pe_rep", name="pe_rep")
                        for kk in range(2):
                            c2 = pg * 2 + kk
                            s, gp = divmod(c2, 6)
                            mv = zht6[s * 64:(s + 1) * 64, gp * R:(gp + 1) * R]
                            stat = sel64[s * 64:(s + 1) * 64,
                                         st * 128:(st + 1) * 128]
                            nc.tensor.matmul(prp[:, kk * 512:kk * 512 + R],
                                             stat, mv, start=True, stop=True,
                                             tile_position=(s * 64, 0))
                        pview = dataclasses.replace(
                            prp[:], ap=[prp[:].ap[0], [512, 2], [1, R]])
                        zsl = ztA[:, pg * 2 * R: pg * 2 * R + 2 * R]
                        bsl = blt[:, pg * 2 * R: pg * 2 * R + 2 * R]
                        osl = dataclasses.replace(
                            bsl, ap=[bsl.ap[0], [R, 2], [1, R]])
                        if pe_i % 12 in (0, 3, 5, 8, 10):
                            nc.vector.tensor_tensor(osl, pview, zsl.rearrange(
                                "p (a f) -> p a f", a=2), mybir.AluOpType.mult)
                        else:
                            bsb = tsbp.tile([128, 2, R], BF16, tag="bsb", name="bsb")
                            nc.scalar.copy(bsb[:], pview)
                            nc.gpsimd.tensor_mul(
                                blt[:, pg * 2 * R: pg * 2 * R + 2 * R],
                                bsb[:].rearrange("p a f -> p (a f)"), zsl)
                    pe_i += 1
                for c2 in range(12):
                    ci = st * 12 + c2
                    nc.tensor.matmul(lt[:], wbc[:, c2, :],
                                     blt[:, c2 * R:(c2 + 1) * R],
                                     start=(ci == 0), stop=(ci == NSTRIP * 12 - 1))

            lout = pp.tile([NL, R], F32, tag="lout", name="lout")
            nc.vector.tensor_scalar_add(lout[:], lt[:], bbs[:, 0:1])
            nc.sync.dma_start(OUT[:], lout[:])
            ps9r_cm.__exit__(None, None, None)
            ps9_cm.__exit__(None, None, None)

    nc.finalize()
    return nc


def _wbt_perm(Wb):
    wbt = Wb.T  # [K, NL]
    order = []
    qv = np.arange(128)
    for st in range(NSTRIP):
        i0 = 2 * st
        for s in range(2):
            for gp in range(6):
                g = 2 * gp + s
                order.append(g * 4096 + (i0 + qv // 64) * 64 + (qv % 64))
    perm = np.concatenate(order)
    return np.ascontiguousarray(wbt[perm]).astype(ml_dtypes.bfloat16)


def _prep_core_inputs(c, sequence_output, attention, mention_mask, Wh, bh, Wt, bt,
                      Wb, bb, mention_idx, hts):
    b, half = c // 2, c % 2
    seq_b = np.ascontiguousarray(sequence_output[b])              # [L, H]
    idx = mention_idx[b].astype(np.int64).reshape(EM)             # [96]
    mask = mention_mask[b].astype(np.float32)                     # [E, M]
    denom = mask.sum(-1)                                          # [E]

    emg = np.ascontiguousarray(seq_b[idx])                        # [96, H]
    amg = np.ascontiguousarray(
        attention[b][:, idx, :].transpose(1, 0, 2).reshape(EM, HL))

    sume = np.zeros((EM, 128), np.float32)
    for e in range(E):
        for m in range(M):
            for rg in range(4):
                sume[e * M + m, rg * 32 + e] = mask[e, m]
    # unused gap partitions: keep exp-sums positive so Ln stays finite
    for rg in range(4):
        sume[0, rg * 32 + E:rg * 32 + 32] = 1.0

    hts_c = hts[b, half * R:(half + 1) * R].astype(np.int64)      # [R, 2]
    ohh = np.zeros((128, R), np.float32)
    oht = np.zeros((128, R), np.float32)
    for rg in range(4):
        ohh[rg * 32 + hts_c[:, 0], np.arange(R)] = 1.0
        oht[rg * 32 + hts_c[:, 1], np.arange(R)] = 1.0

    # ph4 one-hots with mask/denom/sqrt(NH) folded in: [EM, R]
    s = 1.0 / np.sqrt(np.float32(NH))
    wvec = (mask / denom[:, None] * s).reshape(EM)                # [96]
    ohh2 = np.zeros((EM, R), np.float32)
    oht2 = np.zeros((EM, R), np.float32)
    for r in range(R):
        e_h, e_t = hts_c[r, 0], hts_c[r, 1]
        ohh2[e_h * M:(e_h + 1) * M, r] = wvec[e_h * M:(e_h + 1) * M]
        oht2[e_t * M:(e_t + 1) * M, r] = wvec[e_t * M:(e_t + 1) * M]

    # PE-rep one-hots: SEL64[p, st*128+q] = 1 iff p%64 == 2*st + q//64
    sel64 = np.zeros((128, 32 * 128), np.float32)
    for stv in range(32):
        for dq in range(2):
            sel64[2 * stv + dq, stv * 128 + dq * 64:stv * 128 + (dq + 1) * 64] = 1.0
            sel64[64 + 2 * stv + dq, stv * 128 + dq * 64:stv * 128 + (dq + 1) * 64] = 1.0

    bf = ml_dtypes.bfloat16
    return {
        "EMG": emg, "SUME": sume,
        "AMG": amg.astype(bf),
        "OHH": ohh.astype(bf), "OHT": oht.astype(bf),
        "OHH2": ohh2.astype(bf), "OHT2": oht2.astype(bf),
        "SEQ": seq_b.astype(bf),
        "WHT": np.ascontiguousarray(Wh.T).astype(bf),
        "WTT": np.ascontiguousarray(Wt.T).astype(bf),
        "WBT": _wbt_perm(Wb),
        "BHS": np.ascontiguousarray(bh.reshape(6, 128).T),
        "BTS": np.ascontiguousarray(bt.reshape(6, 128).T),
        "BBS": bb.reshape(NL, 1).astype(np.float32),
        "SEL64": sel64.astype(bf),
    }


def kernel(sequence_output, attention, mention_mask, Wh, bh, Wt, bt, Wb, bb,
           mention_idx, hts):
    if "nc" not in _CACHE:
        _CACHE["nc"] = _build_program()
    nc = _CACHE["nc"]

    args = (np.asarray(sequence_output, np.float32), np.asarray(attention, np.float32),
            np.asarray(mention_mask, np.float32), np.asarray(Wh, np.float32),
            np.asarray(bh, np.float32), np.asarray(Wt, np.float32),
            np.asarray(bt, np.float32), np.asarray(Wb, np.float32),
            np.asarray(bb, np.float32), np.asarray(mention_idx),
            np.asarray(hts))
    in_maps = [_prep_core_inputs(c, *args) for c in range(8)]
    try:
        res = run_bass_kernel_spmd(nc, in_maps, list(range(8))).results
    except Exception:
        # transient NRT_EXEC_UNIT_UNRECOVERABLE has been observed on the
        # first execution of a freshly loaded NEFF; retry once
        res = run_bass_kernel_spmd(nc, in_maps, list(range(8))).results

    out = np.empty((B, P, NL), np.float32)
    for c in range(8):
        b, half = c // 2, c % 2
        out[b, half * R:(half + 1) * R, :] = np.asarray(res[c]["OUT"]).T
    return out
